# revision 38
# baseline (speedup 1.0000x reference)
"""Trainium2 Bass kernel for multi-head self-attention (dense transformer block).

Problem: x[4, 2048, 1024], w_qkv[3072, 1024], w_out[1024, 1024], b_out[1024]
  qkv = x @ w_qkv.T, rearranged 'b t (d k h) -> k b h t d' (k=3, h=16)
  attn = softmax(q @ k.T * DIM**-0.5); out = (attn @ v) concat heads @ w_out.T + b_out

Sharding (8 cores): data-parallel over batch b (4) x tensor-parallel over
head-groups (2 groups of 8 heads).  Each core gets x[b].T (bf16, host-cast),
the w_qkv rows for its 8 heads (host-gathered/permuted, bf16), and the
matching w_out columns; it produces TWO partial [T, DIM] f32 outputs (one per
4-head half) which the host sums (the "all-reduce" of the row-sharded w_out
matmul) and adds b_out.

Device-side dataflow per core (engine balance is the point):
  - x stays resident in SBUF (bf16, loaded once); K projected first so the
    exp stream starts ~17us in, Q projected just-in-time per query chunk,
    V/K(heads 4-7)/out-proj spread as backlog filler under a PE-time budget.
  - q/k stored as fp8e4m3 in a [32-partition, 2-slot] per-head layout so the
    64-deep scores contraction runs as DoubleRow matmuls.
  - softmax exp is split between ScalarE (activation Exp, 3 groups/unit) and
    DVE (EXP16_SQ_ANT, a registered custom microcoded op computing
    exp(SCALE*x) by repeated squaring, 1 group/unit) so neither engine
    bottlenecks; the softmax denominator comes free from a ones column
    appended to V.
  - attn is kept bf16; PE-transposed with a bf16 identity; out-projection in
    bf16 per 4-head half, f32 partials DMA'd to two DRAM buffers summed on
    host.
"""

import math
from contextlib import ExitStack
from dataclasses import dataclass

import numpy as np

import concourse.bass as bass
import concourse.mybir as mybir
import concourse.tile as tile
from concourse import bacc
from concourse.bass_utils import run_bass_kernel_spmd
from concourse.masks import make_identity

F32 = mybir.dt.float32
BF16 = mybir.dt.bfloat16
FP8 = mybir.dt.float8e4
DR = mybir.MatmulPerfMode.DoubleRow
EXP = mybir.ActivationFunctionType.Exp
P = 128


@dataclass(frozen=True)
class Cfg:
    T: int = 2048      # sequence length
    DIM: int = 1024    # model dim (= qkv contraction dim)
    NH: int = 8        # heads per core
    DH: int = 64       # head dim
    SCALE: float = 1024.0 ** -0.5

    @property
    def CB(self):      # contraction blocks of 128 over DIM
        return self.DIM // P

    @property
    def OD(self):      # per-core attention width = NH*DH
        return self.NH * self.DH

    @property
    def OB(self):      # o-blocks of 128 (= head pairs, 2 x 64)
        return self.OD // P

    @property
    def JB(self):      # key blocks of 128
        return self.T // P

    @property
    def ICSZ(self):    # query chunk size
        return 256

    @property
    def NIC(self):     # number of query chunks
        return self.T // self.ICSZ

    @property
    def IB(self):      # query blocks of 128 per chunk
        return self.ICSZ // P

    @property
    def TPSZ(self):    # t-chunk of the resident x tiles
        return 256

    @property
    def TPN(self):
        return self.T // self.TPSZ


G2 = 2          # head groups of 4 per core (out-proj halves)
S = 2           # dh slots for the fp8 DoubleRow layout
JJ = 4          # key blocks per scores-PSUM tile / exp instruction
AV_LAG = 3      # units between scores emission and its AV consumption

# exp engine per scores group: ScalarE ("A") or DVE custom op ("D").  GPSIMD
# cannot read PSUM on real hardware, so only these two engines can consume
# scores; 3:1 balances ACT (~199us) against DVE's exp+copy load (~189us).


def exp_eng(u):
    return ("A", "A", "A", "D")

def _register_exp_sq_op():
    """Register a custom microcoded DVE op computing exp(SCALE*x) as
    ((x*c1 + c0)^2 + c2)^16 — squaring-based exp, exactly 8 ALU stages.
    c0=1/sqrt2, c1=SCALE/(16*sqrt2), c2=0.5 give (1 + y + y^2/2)^16 with
    y = SCALE*x/16; rel err <= 1.1e-3 at |SCALE*x|<=2.5.  Uses the standard
    custom-DVE extension point (dve_ops.OPS + per-NEFF uop table); GPSIMD
    cannot read PSUM and the DVE ALU has no native exp, so this is the only
    way to run part of softmax's exp off the Activation engine."""
    import numpy as np_
    from concourse import dve_ops
    from concourse.dve_spec import C0, C1, C2, Spec, Src0, lower, sq
    from concourse.dve_uop import DveOpSpec

    name = "EXP16_SQ_ANT"
    for op in dve_ops.OPS:
        if op.name == name:
            return op

    def ref(in0, in1, s0, s1, imm2):
        r = np_.square(in0.astype(np_.float32) * s1 + s0) + imm2
        for _ in range(4):
            r = r * r
        return r

    body = sq(Src0 * C1 + C0) + C2
    for _ in range(4):
        body = sq(body)
    spec = Spec(body=body, reference=ref)
    opcode = dve_ops._CUSTOM_DVE_ROW_BASE + len(dve_ops.OPS)
    shas = {}
    for ver in ("v3", "v4"):
        compiled = DveOpSpec(
            name=name, opcode=opcode, uops=lower(spec, ver=ver), rd1_en=False
        )
        shas[ver] = compiled.sha(ver)
    op = dve_ops.DveOp(name, spec, subdim=False, uops_sha=shas)
    dve_ops.OPS.append(op)
    dve_ops.CUSTOM_DVE_SPECS[name] = spec
    dve_ops._SUB_OPCODE_FOR_NAME[name] = opcode
    return op


EXP16 = _register_exp_sq_op()
EXP16_C0 = 1.0 / math.sqrt(2.0)
EXP16_C2 = 0.5



def _emit_kernel(tc, cfg, xT, wq, wk, wv, woT, outA, outB):
    nc = tc.nc
    c = cfg
    VW = c.DH + 1  # per-head V width incl. ones column
    NG = c.JB // JJ

    ctx = ExitStack()
    with ctx:
        persist = ctx.enter_context(tc.tile_pool(name="persist", bufs=1))
        xp = ctx.enter_context(tc.tile_pool(name="xp", bufs=c.TPN * c.CB))
        mmp = ctx.enter_context(tc.tile_pool(name="mmp", bufs=2, space="PSUM"))
        smp = ctx.enter_context(tc.tile_pool(name="smp", bufs=4, space="PSUM"))
        ep = ctx.enter_context(tc.tile_pool(name="ep", bufs=6))
        edp = ctx.enter_context(tc.tile_pool(name="edp", bufs=6))
        app = ctx.enter_context(tc.tile_pool(name="app", bufs=6))
        atp = ctx.enter_context(tc.tile_pool(name="atp", bufs=9))
        op = ctx.enter_context(tc.tile_pool(name="op", bufs=3))
        rp = ctx.enter_context(tc.tile_pool(name="rp", bufs=8))

        q8 = persist.tile([P, G2, S, c.T], FP8, name="q8", tag="q8")
        k8 = persist.tile([P, G2, S, c.T], FP8, name="k8", tag="k8")
        v_sb = persist.tile([P, c.JB, c.NH, VW], BF16, name="v_sb", tag="v")
        ident = persist.tile([P, P], BF16, name="ident", tag="ident")
        # wk/wq/wv split into per-cb, per-o-half tiles so the first K matmul
        # starts after ~128KB of DMA and the o-halves not needed until the
        # g2=1 sweep load late.
        HOD = c.OD // 2

        def wtiles(nm):
            return [
                [
                    persist.tile([P, HOD], BF16, name=f"{nm}{cb}_{h}",
                                 tag=f"{nm}{cb}_{h}")
                    for h in range(2)
                ]
                for cb in range(c.CB)
            ]

        wk_sb = wtiles("wk")
        wq_sb = wtiles("wq")
        wv_sb = wtiles("wv")
        woT_sb = [
            persist.tile([P, c.DIM], BF16, name=f"wo{ob}", tag=f"wo{ob}")
            for ob in range(c.OB)
        ]

        make_identity(nc, ident)
        nc.gpsimd.memset(v_sb[:, :, :, c.DH : c.DH + 1], 1.0)

        xT_r = xT.rearrange("(cb p) t -> p cb t", p=P)
        wq_r = wq.rearrange("(cb p) o -> p cb o", p=P)
        wk_r = wk.rearrange("(cb p) o -> p cb o", p=P)
        wv_r = wv.rearrange("(cb p) o -> p cb o", p=P)

        xt = [
            [
                xp.tile([P, c.TPSZ], BF16, name=f"x{tp}_{cb}", tag="x")
                for cb in range(c.CB)
            ]
            for tp in range(c.TPN)
        ]

        def dma_w(w_sb, w_r, half):
            for cb in range(c.CB):
                nc.sync.dma_start(
                    out=w_sb[cb][half], in_=w_r[:, cb, bass.ts(half, HOD)]
                )

        def dma_x(tp):
            for cb in range(c.CB):
                nc.sync.dma_start(
                    out=xt[tp][cb], in_=xT_r[:, cb, bass.ts(tp, c.TPSZ)]
                )

        # DMA order tuned for the pipeline head: wk halves + x[tp0] first
        # (first K matmuls), wq next (JIT q chunk 0), everything else behind.
        dma_w(wk_sb, wk_r, 0)
        dma_x(0)
        dma_x(1)
        dma_w(wq_sb, wq_r, 0)
        for tp in range(2, c.TPN):
            dma_x(tp)
        dma_w(wk_sb, wk_r, 1)
        dma_w(wq_sb, wq_r, 1)
        dma_w(wv_sb, wv_r, 0)
        dma_w(wv_sb, wv_r, 1)
        woT_r = woT.rearrange("(ob p) n -> p ob n", p=P)
        for ob in range(c.OB):
            nc.sync.dma_start(out=woT_sb[ob], in_=woT_r[:, ob, :])

        def emit_k(tp, g2):
            """K projection for x chunk tp, head group g2 (both dh slots)."""
            for s in range(S):
                ob = g2 * 2 + s
                ps = smp.tile([P, c.TPSZ], F32, name="ps_k", tag="sm")
                for cb in range(c.CB):
                    nc.tensor.matmul(
                        ps,
                        wk_sb[cb][g2][:, bass.ts(s, P)],
                        xt[tp][cb],
                        start=(cb == 0),
                        stop=(cb == c.CB - 1),
                    )
                nc.vector.tensor_copy(
                    out=k8[:, g2, s, bass.ts(tp, c.TPSZ)], in_=ps
                )

        def emit_v(tp, g2, tbl):
            """V projection piece: t-block, heads of group g2."""
            ps = smp.tile([P, HOD], F32, name="ps_v", tag="sm")
            for cb in range(c.CB):
                nc.tensor.matmul(
                    ps,
                    xt[tp][cb][:, bass.ts(tbl, P)],
                    wv_sb[cb][g2],
                    start=(cb == 0),
                    stop=(cb == c.CB - 1),
                )
            tb = tp * (c.TPSZ // P) + tbl
            nc.vector.tensor_copy(
                out=v_sb[:, tb, 4 * g2 : 4 * g2 + 4, 0 : c.DH],
                in_=ps.rearrange("p (h d) -> p h d", h=4),
            )

        def emit_q(ic, ob):
            """JIT q projection for query chunk ic, o-block ob=(g2,s)."""
            g2, s = divmod(ob, 2)
            tp, half = divmod(ic, c.TPSZ // c.ICSZ)
            ps = smp.tile([P, c.ICSZ], F32, name="ps_q", tag="sm")
            for cb in range(c.CB):
                nc.tensor.matmul(
                    ps,
                    wq_sb[cb][g2][:, bass.ts(s, P)],
                    xt[tp][cb][:, bass.ts(half, c.ICSZ)],
                    start=(cb == 0),
                    stop=(cb == c.CB - 1),
                )
            nc.vector.tensor_copy(
                out=q8[:, g2, s, bass.ts(ic, c.ICSZ)], in_=ps
            )

        attn_tiles = {}
        attnT_tiles = {}
        stages_done = set()

        def emit_scores_group(g2, ic, hh, g, etiles):
            """One PSUM group (JJ key blocks) of fp8 DoubleRow scores + exp."""
            isl = bass.ts(ic, c.ICSZ)
            pb = 32 * hh
            ps = mmp.tile([P, JJ, c.ICSZ], F32, name="ps_s", tag="mm")
            for jj in range(JJ):
                jb = g * JJ + jj
                nc.tensor.matmul(
                    ps[:, jj, :],
                    k8[pb : pb + 32, g2, :, bass.ts(jb, P)],
                    q8[pb : pb + 32, g2, :, isl],
                    start=True,
                    stop=True,
                    perf_mode=DR,
                    tile_position=(pb, 0),
                )
            e, gg, eng = etiles[g]
            if eng == "A":
                nc.scalar.activation(
                    out=e[:, gg * JJ : (gg + 1) * JJ, :],
                    in_=ps,
                    func=EXP,
                    scale=c.SCALE,
                )
            else:
                nc.vector._custom_dve(
                    EXP16,
                    out=e[:, gg * JJ : (gg + 1) * JJ, :],
                    in0=ps,
                    s0=EXP16_C0,
                    s1=c.SCALE / (16.0 * math.sqrt(2.0)),
                    imm2=EXP16_C2,
                )

        def emit_av(g2, ic, hh, etiles):
            """attn[i, dh] = norm(expST.T @ [v|1]); transpose pair when done."""
            h = 4 * g2 + hh
            hp = h // 2
            key = (ic, hp)
            if hh % 2 == 0:
                attn_tiles[key] = app.tile(
                    [P, c.IB, P], BF16, name=f"attn_{ic}_{hp}", tag="attn"
                )
            attn_sb = attn_tiles[key]
            ps_av = smp.tile([P, c.IB, VW], F32, name="ps_av", tag="sm")
            for ib in range(c.IB):
                for jb in range(c.JB):
                    e, gg, _ = etiles[jb // JJ]
                    jx = gg * JJ + jb % JJ
                    nc.tensor.matmul(
                        ps_av[:, ib, :],
                        e[:, jx, bass.ts(ib, P)],
                        v_sb[:, jb, h, :],
                        start=(jb == 0),
                        stop=(jb == c.JB - 1),
                    )
            rec = rp.tile([P, c.IB], F32, name="rec", tag="rec")
            nc.vector.reciprocal(rec, ps_av[:, :, c.DH])
            for ib in range(c.IB):
                nc.vector.tensor_scalar_mul(
                    out=attn_sb[:, ib, bass.ts(hh % 2, c.DH)],
                    in0=ps_av[:, ib, 0 : c.DH],
                    scalar1=rec[:, ib : ib + 1],
                )
            if hh % 2 == 1:
                if ic not in attnT_tiles:
                    attnT_tiles[ic] = atp.tile(
                        [P, c.OB, c.ICSZ], BF16, name=f"attnT_{ic}", tag="attnT"
                    )
                for ib in range(c.IB):
                    ps_tp = smp.tile([P, P], BF16, name="ps_tp", tag="sm")
                    nc.tensor.transpose(ps_tp, attn_sb[:, ib, :], ident)
                    nc.vector.tensor_copy(
                        out=attnT_tiles[ic][:, hp, bass.ts(ib, P)], in_=ps_tp
                    )
                attn_tiles.pop(key)

        def emit_stage(g2, ic):
            """Out-projection of chunk ic against head-group g2's w_out rows;
            DMA the f32 partial to outA/outB (host sums them)."""
            out_t = outA if g2 == 0 else outB
            attnT = attnT_tiles[ic]
            for tb in range(c.IB):
                o_sb = op.tile([P, c.DIM], F32, name="o_sb", tag="ost")
                for occ in range(2):
                    ps_o = smp.tile([P, c.DIM // 2], F32, name="ps_o", tag="sm")
                    for i, ob in enumerate((2 * g2, 2 * g2 + 1)):
                        nc.tensor.matmul(
                            ps_o,
                            attnT[:, ob, bass.ts(tb, P)],
                            woT_sb[ob][:, bass.ts(occ, c.DIM // 2)],
                            start=(i == 0),
                            stop=(i == 1),
                        )
                    nc.vector.tensor_copy(
                        out=o_sb[:, bass.ts(occ, c.DIM // 2)], in_=ps_o
                    )
                t0 = ic * c.ICSZ + tb * P
                nc.sync.dma_start(out=out_t[t0 : t0 + P, :], in_=o_sb)
            if (1 - g2, ic) in stages_done:
                attnT_tiles.pop(ic)
            stages_done.add((g2, ic))

        # ---- pipeline head: K for heads 0-3, then q chunk 0 ----
        for tp in range(c.TPN):
            emit_k(tp, 0)
            if tp == 4:
                emit_q(0, 0)
                emit_q(0, 1)

        # ---- backlog scheduler ----------------------------------------
        # PE executes its queue in emission order, so scores (which feed the
        # exp engines) are emitted on a tight cadence and all other PE work
        # (AV, V/K/q projections, out-proj) drains from a FIFO backlog under
        # a per-unit PE-time budget.  Arrivals gate work that has a real
        # dependency on a unit's completion (AV pieces, out-proj stages).
        from collections import defaultdict, deque

        arrivals = defaultdict(list)  # unit -> [(cost_ns, fn, args)]
        arrivals_hi = defaultdict(list)  # deadline-critical: jump the queue

        def arrive(u, cost, fn, *args):
            arrivals[u].append((cost, fn, args))

        def arrive_hi(u, cost, fn, *args):
            arrivals_hi[u].append((cost, fn, args))

        order = [
            (g2, ic, hh)
            for g2 in range(G2)
            for ic in range(c.NIC)
            for hh in range(4)
        ]

        vp = [(tp, tbl) for tp in range(c.TPN) for tbl in range(c.TPSZ // P)]
        for i, (tp, tbl) in enumerate(vp):  # V heads 0-3 asap
            arrive(0, 430, emit_v, tp, 0, tbl)
        for i, (tp, tbl) in enumerate(vp):  # V heads 4-7 next
            arrive(6 + i // 2, 430, emit_v, tp, 1, tbl)
        for i, tp in enumerate(range(c.TPN)):  # K heads 4-7
            arrive(14 + 2 * i, 1700, emit_k, tp, 1)
        for ic in range(c.NIC):  # JIT q, o-blocks 0/1 for the g2=0 sweep
            if ic + 1 < c.NIC:
                arrive_hi(4 * ic, 430, emit_q, ic + 1, 0)
                arrive_hi(4 * ic + 1, 430, emit_q, ic + 1, 1)
        for i in range(c.NIC * 2):  # q o-blocks 2/3: ic before its g2=1 use
            ic = i // 2
            uu = 20 + 2 * ic + i % 2 if ic < 5 else 32 + 4 * (ic - 5) + 2
            arrive_hi(uu, 430, emit_q, ic, 2 + (i % 2))
        for ic in range(c.NIC):  # out-proj halves; g2=0 ones deferred into
            # the g2=1 half where the PE has spare capacity
            arrive(33 + (ic * 30) // c.NIC, 2200, emit_stage, 0, ic)
            arrive(32 + 4 * ic + 4 + AV_LAG, 2200, emit_stage, 1, ic)

        NU = 2 * c.NIC * 4
        backlog = deque()

        def drain(budget):
            while backlog and budget > 0:
                cost, fn, args = backlog.popleft()
                fn(*args)
                budget -= cost
            return budget

        AV_COST = 2000  # both AV chains + recip/norm + transposes
        av_pend = {}
        for u in range(NU):
            g2, ic, hh = order[u]
            engs = exp_eng(u)
            e_act = ep.tile(
                [P, 3 * JJ, c.ICSZ], BF16, name=f"ea_{u}", tag="ea"
            )
            e_dve = edp.tile(
                [P, 2 * JJ, c.ICSZ], BF16, name=f"ed_{u}", tag="ed"
            )
            na = nd = 0
            etiles = []
            for g in range(NG):
                if engs[g] == "A":
                    etiles.append((e_act, na, "A"))
                    na += 1
                else:
                    etiles.append((e_dve, nd, "D"))
                    nd += 1
            etiles = tuple(etiles)
            av_pend[u] = (g2, ic, hh, etiles)
            emit_scores_group(g2, ic, hh, 0, etiles)
            emit_scores_group(g2, ic, hh, 1, etiles)
            drain(500)
            emit_scores_group(g2, ic, hh, 2, etiles)
            emit_scores_group(g2, ic, hh, 3, etiles)
            if u - AV_LAG in av_pend:
                backlog.append((AV_COST, emit_av, av_pend.pop(u - AV_LAG)))
            backlog.extend(arrivals.pop(u, ()))
            for piece in reversed(arrivals_hi.pop(u, ())):
                backlog.appendleft(piece)
            drain(2600)
        for u in sorted(av_pend):
            backlog.append((AV_COST, emit_av, av_pend[u]))
        av_pend.clear()
        for u in sorted(arrivals):
            backlog.extend(arrivals[u])
        arrivals.clear()
        drain(10**9)


def build_nc(cfg: Cfg = Cfg(), reps: int = 1):
    nc = bacc.Bacc()
    xT = nc.declare_dram_parameter("xT", [cfg.DIM, cfg.T], BF16, isOutput=False)
    wq = nc.declare_dram_parameter("wq", [cfg.DIM, cfg.OD], BF16, isOutput=False)
    wk = nc.declare_dram_parameter("wk", [cfg.DIM, cfg.OD], BF16, isOutput=False)
    wv = nc.declare_dram_parameter("wv", [cfg.DIM, cfg.OD], BF16, isOutput=False)
    woT = nc.declare_dram_parameter("woT", [cfg.OD, cfg.DIM], BF16, isOutput=False)
    outA = nc.declare_dram_parameter("outA", [cfg.T, cfg.DIM], F32, isOutput=True)
    outB = nc.declare_dram_parameter("outB", [cfg.T, cfg.DIM], F32, isOutput=True)
    with tile.TileContext(nc) as tc:
        for _ in range(reps):
            _emit_kernel(
                tc, cfg, xT[:], wq[:], wk[:], wv[:], woT[:], outA[:], outB[:]
            )
    nc.finalize()
    return nc


def prepare_core_inputs(x, w_qkv, w_out, b, g, cfg: Cfg, n_groups: int):
    """Host-side shard prep for core (batch b, head-group g)."""
    import ml_dtypes

    H = cfg.NH * n_groups
    heads = np.arange(cfg.NH * g, cfg.NH * (g + 1))

    def gather_qk(k_idx):
        # o = ((g2*2 + s)*4 + m)*32 + r; head i = 4*g2 + m, dh = 32*s + r
        o = np.arange(cfg.OD)
        m = (o % P) // 32
        r = o % 32
        g2s = o // P
        g2 = g2s // 2
        s = g2s % 2
        i = 4 * g2 + m
        d = 32 * s + r
        rows = d * (3 * H) + k_idx * H + heads[i]
        return np.ascontiguousarray(w_qkv[rows, :].T).astype(ml_dtypes.bfloat16)

    def gather_v():
        d = np.arange(cfg.DH)
        rows = (d[None, :] * (3 * H) + 2 * H + heads[:, None]).reshape(-1)
        return np.ascontiguousarray(w_qkv[rows, :].T).astype(ml_dtypes.bfloat16)

    return {
        "xT": np.ascontiguousarray(x[b].T).astype(ml_dtypes.bfloat16),
        "wq": gather_qk(0),
        "wk": gather_qk(1),
        "wv": gather_v(),
        "woT": np.ascontiguousarray(
            w_out[:, cfg.OD * g : cfg.OD * (g + 1)].T
        ).astype(ml_dtypes.bfloat16),
    }


_NC_CACHE = {}


def _get_nc(cfg: Cfg):
    if cfg not in _NC_CACHE:
        _NC_CACHE[cfg] = build_nc(cfg)
    return _NC_CACHE[cfg]


def run(x, w_qkv, w_out, b_out, trace=False):
    """Shard, execute on 8 cores, gather. Returns (out, BassKernelResults)."""
    cfg = Cfg()
    B, T, DIM = x.shape
    assert (T, DIM) == (cfg.T, cfg.DIM), (x.shape, cfg)
    n_groups = 2
    nc = _get_nc(cfg)
    in_maps = [
        prepare_core_inputs(x, w_qkv, w_out, b, g, cfg, n_groups)
        for b in range(B)
        for g in range(n_groups)
    ]
    res = run_bass_kernel_spmd(
        nc, in_maps, core_ids=list(range(len(in_maps))), trace=trace
    )
    out = np.empty((B, T, DIM), dtype=np.float32)
    for b in range(B):
        out[b] = (
            res.results[2 * b]["outA"]
            + res.results[2 * b]["outB"]
            + res.results[2 * b + 1]["outA"]
            + res.results[2 * b + 1]["outB"]
        )
    out += b_out.astype(np.float32)
    return out, res


def _make_pjrt_fn(nc, in_maps):
    """Build a non-donating jitted 8-core runner for a prebuilt nc."""
    import jax
    import numpy as np_
    from jax.sharding import Mesh, PartitionSpec
    from jax.experimental.shard_map import shard_map

    from concourse import bass2jax

    bass2jax.install_neuronx_cc_hook()
    n_cores = len(in_maps)
    partition_name = nc.partition_id_tensor.name if nc.partition_id_tensor else None
    in_names, out_names, out_avals, zero_outs = [], [], [], []
    for alloc in nc.m.functions[0].allocations:
        if not isinstance(alloc, mybir.MemoryLocationSet):
            continue
        name = alloc.memorylocations[0].name
        if alloc.kind == "ExternalInput":
            if name != partition_name:
                in_names.append(name)
        elif alloc.kind == "ExternalOutput":
            shape = tuple(alloc.tensor_shape)
            dtype = mybir.dt.np(alloc.dtype)
            out_names.append(name)
            out_avals.append(jax.core.ShapedArray(shape, dtype))
            zero_outs.append(np_.zeros(shape, dtype))
    n_params = len(in_names)
    all_in_names = in_names + out_names
    if partition_name is not None:
        all_in_names = all_in_names + [partition_name]

    def _body(*args):
        operands = list(args)
        if partition_name is not None:
            operands.append(bass2jax.partition_id_tensor())
        return tuple(
            bass2jax._bass_exec_p.bind(
                *operands,
                out_avals=tuple(out_avals),
                in_names=tuple(all_in_names),
                out_names=tuple(out_names),
                lowering_input_output_aliases=(),
                sim_require_finite=True,
                sim_require_nnan=True,
                nc=nc,
            )
        )

    devices = jax.devices()[:n_cores]
    mesh = Mesh(np_.asarray(devices), ("core",))
    nin = n_params + len(out_names)
    f = jax.jit(
        shard_map(
            _body,
            mesh=mesh,
            in_specs=(PartitionSpec("core"),) * nin,
            out_specs=(PartitionSpec("core"),) * len(out_names),
            check_rep=False,
        ),
        keep_unused=True,
    )
    concat_in = [
        np_.concatenate([np_.asarray(in_maps[c][n]) for c in range(n_cores)], axis=0)
        for n in in_names
    ] + [np_.zeros((n_cores * z.shape[0], *z.shape[1:]), z.dtype) for z in zero_outs]
    dev_in = jax.device_put(concat_in)
    return f, dev_in


def _time_fn(f, dev_in, calls=4, rounds=6):
    import time

    import jax

    r = f(*dev_in)
    jax.block_until_ready(r)
    best = float("inf")
    for _ in range(rounds):
        t0 = time.perf_counter()
        rs = [f(*dev_in) for _ in range(calls)]
        jax.block_until_ready(rs)
        best = min(best, (time.perf_counter() - t0) / calls)
    return best


def time_hw(x, w_qkv, w_out, b_out, reps=(4, 36)):
    """Marginal-cost HW timing: per-call time of an R2-repeat NEFF minus an
    R1-repeat NEFF, over (R2-R1), cancels the axon dispatch overhead."""
    cfg = Cfg()
    B = x.shape[0]
    in_maps = [
        prepare_core_inputs(x, w_qkv, w_out, b, g, cfg, 2)
        for b in range(B)
        for g in range(2)
    ]
    r1, r2 = reps
    ncA = build_nc(cfg, reps=r1)
    fA, devA = _make_pjrt_fn(ncA, in_maps)
    tA = _time_fn(fA, devA)
    ncB = build_nc(cfg, reps=r2)
    fB, devB = _make_pjrt_fn(ncB, in_maps)
    tB = _time_fn(fB, devB)
    per_exec = (tB - tA) / (r2 - r1)
    return tA, per_exec


def kernel(x, w_qkv, w_out, b_out):
    x = np.asarray(x, dtype=np.float32)
    w_qkv = np.asarray(w_qkv, dtype=np.float32)
    w_out = np.asarray(w_out, dtype=np.float32)
    b_out = np.asarray(b_out, dtype=np.float32)
    try:
        out, _ = run(x, w_qkv, w_out, b_out, trace=False)
    except Exception:
        # one retry for transient device errors
        out, _ = run(x, w_qkv, w_out, b_out, trace=False)
    return out
